# revision 4
# baseline (speedup 1.0000x reference)
"""Trainium2 Bass kernel for nn_Average_Model_fwRF.

The whole model is a single linear functional of the inputs:

    out[b] = sum_l <fmap_l[b], mass_l (x) W_l> + s * sum(fc gathers * W) + bias
           = <X[b, :], V> + bias

so we fold the Gaussian masses and the [1,4200] linear weight into one
vector V (host side, tiny), pack each core's 64-batch slice of all the
fmaps/fcs into a d-major layout, and the device kernel is a streaming
dot product: for each [128 x 512] tile, matmul(lhsT=V-tile[128,8],
rhs=X-tile[128,512]) accumulating into one PSUM bank.  The 512-wide
free dim packs 8 d-groups x 64 batch; only the "diagonal" (group g of
the output row g) is extracted at the end.

Pure data parallel over batch: 8 cores x 64 batch, no collectives.
"""

import sys

if "/opt/trn_rl_repo" not in sys.path:
    sys.path.insert(0, "/opt/trn_rl_repo")

import numpy as np

B = 512
N_CORES = 8
BPC = B // N_CORES  # 64 batch per core
CONV = [(64, 27), (192, 27), (384, 13), (256, 13), (256, 13)]
FC_MAX = 1024
FC2 = 1000

D_CONV = sum(c * h * h for c, h in CONV)  # 338048
D_RAW = D_CONV + FC_MAX + FC_MAX + FC2  # 341096

G = 8  # d-groups per matmul; rhs free dim = G*BPC = 512
FREE = G * BPC  # 512
D_PER_MM = G * 128  # 1024 d-values per matmul
NMM = -(-D_RAW // D_PER_MM)  # 334
TPC = 8  # matmuls per DMA chunk (chunk = 2 MiB)
NMM = -(-NMM // TPC) * TPC  # 336, pad to whole chunks
NCHUNK = NMM // TPC  # 42
DP = NMM * D_PER_MM  # 344064 padded feature dim

PROFILE = False  # set by test.py (needs the ntff shim installed)
_CACHE = {}


def _build():
    import concourse.tile as tile
    from concourse import bacc, mybir

    nc = bacc.Bacc("TRN2", debug=False, num_devices=N_CORES, enable_asserts=False)
    xt_d = nc.dram_tensor("xt", [128, NMM * FREE], mybir.dt.float32r,
                          kind="ExternalInput")
    vt_d = nc.dram_tensor("vt", [128, NMM * G], mybir.dt.float32r,
                          kind="ExternalInput")
    out_d = nc.dram_tensor("o8", [G, FREE], mybir.dt.float32,
                           kind="ExternalOutput")

    with tile.TileContext(nc) as tc:
        with (
            tc.tile_pool(name="vp", bufs=1) as vp,
            tc.tile_pool(name="xp", bufs=3) as xp,
            tc.tile_pool(name="pp", bufs=1, space="PSUM") as pp,
            tc.tile_pool(name="op", bufs=1) as op,
        ):
            vt = vp.tile([128, NMM * G], mybir.dt.float32r)
            nc.sync.dma_start(vt[:], vt_d.ap()[:])
            ps = pp.tile([G, FREE], mybir.dt.float32)
            for c in range(NCHUNK):
                xt = xp.tile([128, TPC * FREE], mybir.dt.float32r)
                nc.sync.dma_start(
                    xt[:], xt_d.ap()[:, c * TPC * FREE:(c + 1) * TPC * FREE]
                )
                for q in range(TPC):
                    t = c * TPC + q
                    nc.tensor.matmul(
                        ps[:],
                        vt[:, t * G:(t + 1) * G],
                        xt[:, q * FREE:(q + 1) * FREE],
                        start=(t == 0),
                        stop=(t == NMM - 1),
                    )
            o8 = op.tile([G, FREE], mybir.dt.float32)
            nc.vector.tensor_copy(o8[:], ps[:])
            nc.sync.dma_start(out_d.ap()[:], o8[:])

    nc.compile()
    return nc


def _build_v(mass, mfc, W, idx0, idx1):
    """Fold masses, the fc scalar, and W into one length-DP vector."""
    W = np.asarray(W, dtype=np.float32).reshape(-1)
    s = np.float32(np.asarray(mfc).reshape(-1)[0])
    v = np.zeros(DP, dtype=np.float32)
    off_w = 0
    off_d = 0
    for (c, h), m in zip(CONV, mass):
        m = np.asarray(m, dtype=np.float32)
        v[off_d:off_d + c * h * h] = (
            W[off_w:off_w + c, None, None] * m[None, :, :]
        ).reshape(-1)
        off_w += c
        off_d += c * h * h
    for n in (FC_MAX, FC_MAX, FC2):
        v[off_d:off_d + n] = s * W[off_w:off_w + n]
        off_w += n
        off_d += n
    return v


def _pack_x(fmaps, fc0, fc1, fc2, idx0, idx1):
    """[B, D_RAW] activations -> per-core [128, NMM*FREE] d-major layout.

    Layout: col = t*FREE + g*64 + b, partition = p, holding
    X[core*64 + b, (t*G + g)*128 + p].
    """
    xall = np.zeros((B, DP), dtype=np.float32)
    off = 0
    for f in fmaps:
        f = np.asarray(f, dtype=np.float32)
        n = f.shape[1] * f.shape[2] * f.shape[3]
        xall[:, off:off + n] = f.reshape(B, n)
        off += n
    xall[:, off:off + FC_MAX] = np.asarray(fc0, dtype=np.float32).reshape(B, -1)[:, idx0]
    off += FC_MAX
    xall[:, off:off + FC_MAX] = np.asarray(fc1, dtype=np.float32).reshape(B, -1)[:, idx1]
    off += FC_MAX
    xall[:, off:off + FC2] = np.asarray(fc2, dtype=np.float32).reshape(B, -1)
    off += FC2
    assert off == D_RAW

    # [core, b, t, g, p] -> [core, p, t, g, b]
    a = xall.reshape(N_CORES, BPC, NMM, G, 128)
    a = np.ascontiguousarray(a.transpose(0, 4, 2, 3, 1))
    return a.reshape(N_CORES, 128, NMM * FREE)


def kernel(fmap0, fmap1, fmap2, fmap3, fmap4, fc0, fc1, fc2,
           mass0, mass1, mass2, mass3, mass4, mfc, W, b, idx0, idx1):
    from concourse.bass_utils import run_bass_kernel_spmd

    if "nc" not in _CACHE:
        _CACHE["nc"] = _build()
    nc = _CACHE["nc"]

    idx0 = np.asarray(idx0).astype(np.int64)
    idx1 = np.asarray(idx1).astype(np.int64)

    v = _build_v([mass0, mass1, mass2, mass3, mass4], mfc, W, idx0, idx1)
    vh = np.ascontiguousarray(v.reshape(NMM * G, 128).T)  # [128, NMM*G]

    xh = _pack_x([fmap0, fmap1, fmap2, fmap3, fmap4], fc0, fc1, fc2, idx0, idx1)

    in_maps = [{"xt": xh[i], "vt": vh} for i in range(N_CORES)]

    res = run_bass_kernel_spmd(
        nc, in_maps, core_ids=list(range(N_CORES)), trace=PROFILE
    )
    if PROFILE and res.exec_time_ns is not None:
        print(f"HW exec time: {res.exec_time_ns} ns")
        _CACHE["exec_time_ns"] = res.exec_time_ns
        _CACHE["trace"] = res.instructions_and_trace

    bias = np.float32(np.asarray(b).reshape(-1)[0])
    out = np.empty((B, 1), dtype=np.float32)
    for i in range(N_CORES):
        o8 = res.results[i]["o8"].reshape(G, G, BPC)  # [g, g', b]
        diag = o8[np.arange(G), np.arange(G)]  # [G, BPC]
        out[i * BPC:(i + 1) * BPC, 0] = diag.sum(axis=0, dtype=np.float32) + bias
    return out


# revision 10
# speedup vs baseline: 1.3481x; 1.3481x over previous
"""Trainium2 Bass kernel for nn_Average_Model_fwRF.

The whole model is a single linear functional of the inputs:

    out[b] = sum_l <fmap_l[b], mass_l (x) W_l> + s * sum(fc gathers * W) + bias
           = <X[b, :], V> + bias

so we fold the Gaussian masses and the [1,4200] linear weight into one
vector V (host side, tiny), pack each core's 64-batch slice of all the
fmaps/fcs into a d-major layout, and the device kernel is a streaming
dot product: for each [128 x 512] tile, matmul(lhsT=V-tile[128,8],
rhs=X-tile[128,512]) accumulating into one PSUM bank.  The 512-wide
free dim packs 8 d-groups x 64 batch; only the "diagonal" (group g of
the output row g) is extracted at the end.

Pure data parallel over batch: 8 cores x 64 batch, no collectives.
"""

import sys

if "/opt/trn_rl_repo" not in sys.path:
    sys.path.insert(0, "/opt/trn_rl_repo")

import numpy as np

B = 512
N_CORES = 8
BPC = B // N_CORES  # 64 batch per core
CONV = [(64, 27), (192, 27), (384, 13), (256, 13), (256, 13)]
FC_MAX = 1024
FC2 = 1000

D_CONV = sum(c * h * h for c, h in CONV)  # 338048
D_RAW = D_CONV + FC_MAX + FC_MAX + FC2  # 341096

G = 8  # d-groups per matmul; rhs free dim = G*BPC = 512
FREE = G * BPC  # 512
D_PER_MM = G * 128  # 1024 d-values per matmul
NMM = -(-D_RAW // D_PER_MM)  # 334
TPC = 8  # matmuls per DMA chunk (chunk = 2 MiB)
NMM = -(-NMM // TPC) * TPC  # 336, pad to whole chunks
NCHUNK = NMM // TPC  # 42
DP = NMM * D_PER_MM  # 344064 padded feature dim

PROFILE = False  # set by test.py (needs the ntff shim installed)
DTYPE = "float16"  # "float16" or "float32r" (device staging/matmul dtype)
VSCALE = np.float32(512.0)  # fp16: V pre-scaled by 2^9 to dodge subnormals
_CACHE = {}


def _np_dtype():
    return np.float16 if DTYPE == "float16" else np.float32


def _build():
    import concourse.tile as tile
    from concourse import bacc, mybir

    dt = getattr(mybir.dt, DTYPE)
    nc = bacc.Bacc("TRN2", debug=False, num_devices=N_CORES, enable_asserts=False)
    xt_d = nc.dram_tensor("xt", [128, NMM * FREE], dt, kind="ExternalInput")
    vt_d = nc.dram_tensor("vt", [128, NMM * G], dt, kind="ExternalInput")
    out_d = nc.dram_tensor("o8", [G, FREE], mybir.dt.float32,
                           kind="ExternalOutput")

    with tile.TileContext(nc) as tc:
        with (
            tc.tile_pool(name="vp", bufs=1) as vp,
            tc.tile_pool(name="xp", bufs=3) as xp,
            tc.tile_pool(name="pp", bufs=1, space="PSUM") as pp,
            tc.tile_pool(name="op", bufs=1) as op,
        ):
            vt = vp.tile([128, NMM * G], dt)
            nc.sync.dma_start(vt[:], vt_d.ap()[:])
            ps = pp.tile([G, FREE], mybir.dt.float32)
            for c in range(NCHUNK):
                xt = xp.tile([128, TPC * FREE], dt)
                nc.sync.dma_start(
                    xt[:], xt_d.ap()[:, c * TPC * FREE:(c + 1) * TPC * FREE]
                )
                for q in range(TPC):
                    t = c * TPC + q
                    nc.tensor.matmul(
                        ps[:],
                        vt[:, t * G:(t + 1) * G],
                        xt[:, q * FREE:(q + 1) * FREE],
                        start=(t == 0),
                        stop=(t == NMM - 1),
                    )
            o8 = op.tile([G, FREE], mybir.dt.float32)
            nc.vector.tensor_copy(o8[:], ps[:])
            nc.sync.dma_start(out_d.ap()[:], o8[:])

    nc.compile()
    return nc


def _scale():
    return VSCALE if DTYPE == "float16" else np.float32(1.0)


def _build_v(mass, mfc, W, idx0, idx1):
    """Fold masses, the fc scalar, and W into one length-DP vector."""
    W = np.asarray(W, dtype=np.float32).reshape(-1) * _scale()
    s = np.float32(np.asarray(mfc).reshape(-1)[0])
    v = np.zeros(DP, dtype=np.float32)
    off_w = 0
    off_d = 0
    for (c, h), m in zip(CONV, mass):
        m = np.asarray(m, dtype=np.float32)
        v[off_d:off_d + c * h * h] = (
            W[off_w:off_w + c, None, None] * m[None, :, :]
        ).reshape(-1)
        off_w += c
        off_d += c * h * h
    for n in (FC_MAX, FC_MAX, FC2):
        v[off_d:off_d + n] = s * W[off_w:off_w + n]
        off_w += n
        off_d += n
    return v


def _pack_x(fmaps, fc0, fc1, fc2, idx0, idx1):
    """[B, D_RAW] activations -> per-core [128, NMM*FREE] d-major layout.

    Layout: col = t*FREE + g*64 + b, partition = p, holding
    X[core*64 + b, (t*G + g)*128 + p].
    """
    xall = np.zeros((B, DP), dtype=_np_dtype())
    off = 0
    for f in fmaps:
        f = np.asarray(f, dtype=np.float32)
        n = f.shape[1] * f.shape[2] * f.shape[3]
        xall[:, off:off + n] = f.reshape(B, n)
        off += n
    xall[:, off:off + FC_MAX] = np.asarray(fc0, dtype=np.float32).reshape(B, -1)[:, idx0]
    off += FC_MAX
    xall[:, off:off + FC_MAX] = np.asarray(fc1, dtype=np.float32).reshape(B, -1)[:, idx1]
    off += FC_MAX
    xall[:, off:off + FC2] = np.asarray(fc2, dtype=np.float32).reshape(B, -1)
    off += FC2
    assert off == D_RAW

    # [core, b, t, g, p] -> [core, p, t, g, b]
    a = xall.reshape(N_CORES, BPC, NMM, G, 128)
    a = np.ascontiguousarray(a.transpose(0, 4, 2, 3, 1))
    return a.reshape(N_CORES, 128, NMM * FREE)


def kernel(fmap0, fmap1, fmap2, fmap3, fmap4, fc0, fc1, fc2,
           mass0, mass1, mass2, mass3, mass4, mfc, W, b, idx0, idx1):
    from concourse.bass_utils import run_bass_kernel_spmd

    if "nc" not in _CACHE:
        _CACHE["nc"] = _build()
    nc = _CACHE["nc"]

    idx0 = np.asarray(idx0).astype(np.int64)
    idx1 = np.asarray(idx1).astype(np.int64)

    v = _build_v([mass0, mass1, mass2, mass3, mass4], mfc, W, idx0, idx1)
    vh = np.ascontiguousarray(v.reshape(NMM * G, 128).T.astype(_np_dtype()))

    xh = _pack_x([fmap0, fmap1, fmap2, fmap3, fmap4], fc0, fc1, fc2, idx0, idx1)

    in_maps = [{"xt": xh[i], "vt": vh} for i in range(N_CORES)]

    res = run_bass_kernel_spmd(
        nc, in_maps, core_ids=list(range(N_CORES)), trace=PROFILE
    )
    if PROFILE and res.exec_time_ns is not None:
        print(f"HW exec time: {res.exec_time_ns} ns")
        _CACHE["exec_time_ns"] = res.exec_time_ns
        _CACHE["trace"] = res.instructions_and_trace

    bias = np.float32(np.asarray(b).reshape(-1)[0])
    inv_scale = np.float32(1.0) / _scale()
    out = np.empty((B, 1), dtype=np.float32)
    for i in range(N_CORES):
        o8 = res.results[i]["o8"].reshape(G, G, BPC)  # [g, g', b]
        diag = o8[np.arange(G), np.arange(G)]  # [G, BPC]
        out[i * BPC:(i + 1) * BPC, 0] = (
            diag.sum(axis=0, dtype=np.float32) * inv_scale + bias
        )
    return out


# revision 15
# speedup vs baseline: 1.6312x; 1.2100x over previous
"""Trainium2 Bass kernel for nn_Average_Model_fwRF.

The whole model is a single linear functional of the inputs:

    out[b] = sum_l <fmap_l[b], mass_l (x) W_l> + s * sum(fc gathers * W) + bias
           = <X[b, :], V> + bias

so we fold the Gaussian masses and the [1,4200] linear weight into one
vector V (host side, tiny), pack each core's 64-batch slice of all the
fmaps/fcs into a d-major layout, and the device kernel is a streaming
dot product: for each [128 x 512] tile, matmul(lhsT=V-tile[128,8],
rhs=X-tile[128,512]) accumulating into one PSUM bank.  The 512-wide
free dim packs 8 d-groups x 64 batch; only the "diagonal" (group g of
the output row g) is extracted at the end.

Pure data parallel over batch: 8 cores x 64 batch, no collectives.
"""

import sys

if "/opt/trn_rl_repo" not in sys.path:
    sys.path.insert(0, "/opt/trn_rl_repo")

import numpy as np

B = 512
N_CORES = 8
BPC = B // N_CORES  # 64 batch per core
CONV = [(64, 27), (192, 27), (384, 13), (256, 13), (256, 13)]
FC_MAX = 1024
FC2 = 1000

D_CONV = sum(c * h * h for c, h in CONV)  # 338048
D_RAW = D_CONV + FC_MAX + FC_MAX + FC2  # 341096

G = 8  # d-groups per matmul; rhs free dim = G*BPC = 512
FREE = G * BPC  # 512
TW = G + FREE  # 520 columns per tile in the interleaved stream (8 V + 512 X)
D_PER_MM = G * 128  # 1024 d-values per matmul
NMM = -(-D_RAW // D_PER_MM)  # 334
TPC = 16  # matmuls per DMA chunk (chunk = ~2 MiB fp16)
NMM = -(-NMM // TPC) * TPC  # 336, pad to whole chunks
NCHUNK = NMM // TPC  # 21
DP = NMM * D_PER_MM  # 344064 padded feature dim
XBUFS = 5  # SBUF chunk buffers (deep prefetch keeps DMA from stalling)
WARM_MM = 12  # PE warm-up matmuls on scratch data at kernel start

PROFILE = False  # set by test.py (needs the ntff shim installed)
DTYPE = "float16"  # "float16" or "float32r" (device staging/matmul dtype)
VSCALE = np.float32(512.0)  # fp16: V pre-scaled by 2^9 to dodge subnormals
_CACHE = {}


def _np_dtype():
    return np.float16 if DTYPE == "float16" else np.float32


def _build():
    import concourse.tile as tile
    from concourse import bacc, mybir

    dt = getattr(mybir.dt, DTYPE)
    nc = bacc.Bacc("TRN2", debug=False, num_devices=N_CORES, enable_asserts=False)
    xv_d = nc.dram_tensor("xv", [128, NMM * TW], dt, kind="ExternalInput")
    out_d = nc.dram_tensor("o8", [G, FREE], mybir.dt.float32,
                           kind="ExternalOutput")

    with tile.TileContext(nc) as tc:
        with (
            tc.tile_pool(name="wp", bufs=1) as wp,
            tc.tile_pool(name="xp", bufs=XBUFS) as xp,
            tc.tile_pool(name="pp", bufs=1, space="PSUM") as pp,
            tc.tile_pool(name="wq", bufs=1, space="PSUM") as wq,
            tc.tile_pool(name="op", bufs=1) as op,
        ):
            # PE warm-up: ~5us of matmuls on scratch data so HAM reaches
            # K=8/8 while the first chunks are still in flight.
            wt = wp.tile([128, TW], dt)
            nc.gpsimd.memset(wt[:], 0.0)
            wps = wq.tile([G, FREE], mybir.dt.float32)
            for _ in range(WARM_MM):
                nc.tensor.matmul(wps[:], wt[:, :G], wt[:, G:], start=True,
                                 stop=True)

            ps = pp.tile([G, FREE], mybir.dt.float32)
            for c in range(NCHUNK):
                xt = xp.tile([128, TPC * TW], dt)
                nc.sync.dma_start(
                    xt[:], xv_d.ap()[:, c * TPC * TW:(c + 1) * TPC * TW]
                )
                for q in range(TPC):
                    t = c * TPC + q
                    nc.tensor.matmul(
                        ps[:],
                        xt[:, q * TW:q * TW + G],
                        xt[:, q * TW + G:(q + 1) * TW],
                        start=(t == 0),
                        stop=(t == NMM - 1),
                    )
            o8 = op.tile([G, FREE], mybir.dt.float32)
            nc.vector.tensor_copy(o8[:], ps[:])
            nc.sync.dma_start(out_d.ap()[:], o8[:])

    nc.compile()
    return nc


def _scale():
    return VSCALE if DTYPE == "float16" else np.float32(1.0)


def _build_v(mass, mfc, W, idx0, idx1):
    """Fold masses, the fc scalar, and W into one length-DP vector."""
    W = np.asarray(W, dtype=np.float32).reshape(-1) * _scale()
    s = np.float32(np.asarray(mfc).reshape(-1)[0])
    v = np.zeros(DP, dtype=np.float32)
    off_w = 0
    off_d = 0
    for (c, h), m in zip(CONV, mass):
        m = np.asarray(m, dtype=np.float32)
        v[off_d:off_d + c * h * h] = (
            W[off_w:off_w + c, None, None] * m[None, :, :]
        ).reshape(-1)
        off_w += c
        off_d += c * h * h
    for n in (FC_MAX, FC_MAX, FC2):
        v[off_d:off_d + n] = s * W[off_w:off_w + n]
        off_w += n
        off_d += n
    return v


def _pack_x(fmaps, fc0, fc1, fc2, idx0, idx1):
    """[B, D_RAW] activations -> per-core [128, NMM*FREE] d-major layout.

    Layout: col = t*FREE + g*64 + b, partition = p, holding
    X[core*64 + b, (t*G + g)*128 + p].
    """
    xall = np.zeros((B, DP), dtype=_np_dtype())
    off = 0
    for f in fmaps:
        f = np.asarray(f, dtype=np.float32)
        n = f.shape[1] * f.shape[2] * f.shape[3]
        xall[:, off:off + n] = f.reshape(B, n)
        off += n
    xall[:, off:off + FC_MAX] = np.asarray(fc0, dtype=np.float32).reshape(B, -1)[:, idx0]
    off += FC_MAX
    xall[:, off:off + FC_MAX] = np.asarray(fc1, dtype=np.float32).reshape(B, -1)[:, idx1]
    off += FC_MAX
    xall[:, off:off + FC2] = np.asarray(fc2, dtype=np.float32).reshape(B, -1)
    off += FC2
    assert off == D_RAW

    # [core, b, t, g, p] -> [core, p, t, g, b]
    return xall.reshape(N_CORES, BPC, NMM, G, 128).transpose(0, 4, 2, 3, 1)


def kernel(fmap0, fmap1, fmap2, fmap3, fmap4, fc0, fc1, fc2,
           mass0, mass1, mass2, mass3, mass4, mfc, W, b, idx0, idx1):
    from concourse.bass_utils import run_bass_kernel_spmd

    if "nc" not in _CACHE:
        _CACHE["nc"] = _build()
    nc = _CACHE["nc"]

    idx0 = np.asarray(idx0).astype(np.int64)
    idx1 = np.asarray(idx1).astype(np.int64)

    v = _build_v([mass0, mass1, mass2, mass3, mass4], mfc, W, idx0, idx1)
    vh = v.reshape(NMM, G, 128).transpose(2, 0, 1).astype(_np_dtype())  # [p,t,g]

    xh = _pack_x([fmap0, fmap1, fmap2, fmap3, fmap4], fc0, fc1, fc2, idx0, idx1)

    # interleaved stream: per tile t, 8 V columns then 512 X columns
    xv = np.empty((N_CORES, 128, NMM, TW), dtype=_np_dtype())
    xv[:, :, :, :G] = vh[None]
    for g in range(G):
        xv[:, :, :, G + g * BPC:G + (g + 1) * BPC] = xh[:, :, :, g, :]
    xv = xv.reshape(N_CORES, 128, NMM * TW)

    in_maps = [{"xv": xv[i]} for i in range(N_CORES)]

    res = run_bass_kernel_spmd(
        nc, in_maps, core_ids=list(range(N_CORES)), trace=PROFILE
    )
    if PROFILE and res.exec_time_ns is not None:
        print(f"HW exec time: {res.exec_time_ns} ns")
        _CACHE["exec_time_ns"] = res.exec_time_ns
        _CACHE["trace"] = res.instructions_and_trace

    bias = np.float32(np.asarray(b).reshape(-1)[0])
    inv_scale = np.float32(1.0) / _scale()
    out = np.empty((B, 1), dtype=np.float32)
    for i in range(N_CORES):
        o8 = res.results[i]["o8"].reshape(G, G, BPC)  # [g, g', b]
        diag = o8[np.arange(G), np.arange(G)]  # [G, BPC]
        out[i * BPC:(i + 1) * BPC, 0] = (
            diag.sum(axis=0, dtype=np.float32) * inv_scale + bias
        )
    return out
